# revision 28
# baseline (speedup 1.0000x reference)
"""MoE transformer layer on 8 Trainium2 NeuronCores.

Strategy:
  Launch 1 (attention): shard by (batch, head-group) -> 8 cores.
    Core (b, g) holds all 1024 tokens of batch b and computes LN1 ->
    Q/K/V for its 8 heads -> softmax -> AV -> its partial of the output
    projection, all in bf16 with features on partitions. No K/V
    duplication across cores; LN1 gain/bias are folded into the QKV
    weights on the host. Output: partial attn projection [E, S] bf16.
  Host: combine the two partials per batch + residual -> x2; LN2 ->
    h2; top-2 gating (softmax over 8 logits, renormalized); builds the
    per-expert token batches (all-to-all dispatch done on host).
  Launch 2 (expert FFN): expert-parallel, core e owns expert e.
    toksT [E, C] bf16 -> gelu(w1.T @ toks + b1) -> w2.T @ h + b2.
  Host: scatter-add combine with gate weights + residual.
"""

import numpy as np

import concourse.bass as bass
import concourse.tile as tile
from concourse import bacc, mybir
from concourse.bass_utils import run_bass_kernel_spmd

S, B, E = 1024, 4, 1024
H, DH = 16, 64
F, NE = 4096, 8
N = S * B
NCORES = 8
C = 1088         # expert capacity (max expert load for seed-0 inputs is ~1076)
CT = [(0, 512), (512, 512), (1024, 64)]  # (offset, width) token tiles in launch 2
ET = E // 128    # 8
FT = F // 128    # 32
NP = 4           # head pairs per core (8 heads)

f32 = mybir.dt.float32
f32r = mybir.dt.float32r
bf16 = mybir.dt.bfloat16
AF = mybir.ActivationFunctionType
ALU = mybir.AluOpType

_GELU = AF.Gelu

_programs = {}


def _build_launch1():
    nc = bacc.Bacc("TRN2", target_bir_lowering=False, debug=False, num_devices=NCORES)

    xT_d = nc.dram_tensor("xT", [E, S], bf16, kind="ExternalInput").ap()
    wqkv_d = nc.dram_tensor("wqkv", [E, 1536], bf16, kind="ExternalInput").ap()
    bqkv_d = nc.dram_tensor("bqkv", [8 * 128, 1], f32, kind="ExternalInput").ap()
    woT_d = nc.dram_tensor("woT", [512, E], bf16, kind="ExternalInput").ap()
    sel2_d = nc.dram_tensor("sel2", [2, 128], f32, kind="ExternalInput").ap()
    ident_d = nc.dram_tensor("ident", [128, 128], bf16, kind="ExternalInput").ap()
    outT_d = nc.dram_tensor("outT", [E, S], bf16, kind="ExternalOutput").ap()

    with tile.TileContext(nc) as tc:
        consts = tc.alloc_tile_pool(name="consts", bufs=1)
        statp = tc.alloc_tile_pool(name="stat", bufs=1)
        bcp = tc.alloc_tile_pool(name="bc", bufs=1)
        sqp = tc.alloc_tile_pool(name="sqp", bufs=2)
        wsp = tc.alloc_tile_pool(name="wstream", bufs=1)
        qkvp = tc.alloc_tile_pool(name="qkvp", bufs=2)
        vsp = tc.alloc_tile_pool(name="vsp", bufs=1)
        attnp = tc.alloc_tile_pool(name="attnp", bufs=12)
        otp = tc.alloc_tile_pool(name="otp", bufs=1)
        outp = tc.alloc_tile_pool(name="outp", bufs=3)
        pmm = tc.alloc_tile_pool(name="pmm", bufs=2, space="PSUM")
        psc = tc.alloc_tile_pool(name="psc", bufs=2, space="PSUM")
        pav = tc.alloc_tile_pool(name="pav", bufs=2, space="PSUM")

        ones128 = consts.tile([128, 1], bf16, tag="ones128")
        nc.vector.memset(ones128[:], 1.0)
        ones1b = consts.tile([1, 128], bf16, tag="ones1b")
        nc.vector.memset(ones1b[:], 1.0)
        ones1f = consts.tile([1, 128], f32r, tag="ones1f")
        nc.vector.memset(ones1f[:].bitcast(f32), 1.0)
        eps = consts.tile([1, 1], f32, tag="eps")
        nc.vector.memset(eps[:], 1e-5)
        dust = consts.tile([1, 1], f32, tag="dust")

        # prime the ACT Ln/Exp table set right away
        nc.scalar.activation(out=dust[:], in_=eps[:], func=AF.Ln, scale=1.0)
        nc.scalar.activation(out=dust[:], in_=dust[:], func=AF.Exp, scale=1.0)

        ident = consts.tile([128, 128], bf16, tag="ident")
        nc.sync.dma_start(out=ident[:], in_=ident_d)
        sel_h = []
        for h in range(2):
            st = consts.tile([1, 128], f32r, tag=f"sel{h}")
            nc.sync.dma_start(out=st[:], in_=sel2_d[h:h + 1, :].bitcast(f32r))
            sel_h.append(st)
        bqkv_sb = consts.tile([128, 8], f32, tag="bqkv")
        nc.sync.dma_start(out=bqkv_sb[:],
                          in_=bqkv_d.rearrange("(a p) o -> p (a o)", p=128))

        # ---------- phase 1: load x (bf16, column-split), LN1 stats ----------
        lxp = tc.alloc_tile_pool(name="lxp", bufs=1)
        xp = tc.alloc_tile_pool(name="xp", bufs=1)

        xbig = xp.tile([128, ET * S], bf16, tag="x", name="x_sb")
        xT_r = xT_d.rearrange("(a p) c -> p a c", p=128)
        xbig_r = xbig[:].rearrange("p (a c) -> p a c", a=ET)
        nc.sync.dma_start(out=xbig_r[:, :, 0:512], in_=xT_r[:, :, 0:512])
        nc.scalar.dma_start(out=xbig_r[:, :, 512:1024], in_=xT_r[:, :, 512:1024])
        x_sb = [xbig[:, i * S:(i + 1) * S] for i in range(ET)]

        # all qkv weights resident; three engine-parallel DMAs
        w_sb = wsp.tile([128, ET * 1536], bf16, tag="wqkv", name="w_sb")
        w_r = w_sb[:].rearrange("p (a c) -> p a c", a=ET)
        wd_r = wqkv_d.rearrange("(a p) c -> p a c", p=128)
        nc.scalar.dma_start(out=w_r[:, :, 0:512], in_=wd_r[:, :, 0:512])
        nc.gpsimd.dma_start(out=w_r[:, :, 512:1024], in_=wd_r[:, :, 512:1024])
        nc.sync.dma_start(out=w_r[:, :, 1024:1536], in_=wd_r[:, :, 1024:1536])

        def wq_t(p, kt):
            return w_sb[:, kt * 1536 + p * 128: kt * 1536 + (p + 1) * 128]

        def wk_t(p, kt):
            return w_sb[:, kt * 1536 + 512 + p * 128: kt * 1536 + 512 + (p + 1) * 128]

        def wv_t(kt):
            return w_sb[:, kt * 1536 + 1024: kt * 1536 + 1536]

        # out-proj weights: one early DMA
        wo_big = wsp.tile([128, 4 * E], bf16, tag="wo", name="wo")
        nc.gpsimd.dma_start(
            out=wo_big[:].rearrange("p (a c) -> p a c", a=4),
            in_=woT_d.rearrange("(a p) c -> p a c", p=128))
        wo_t = [wo_big[:, ft * E:(ft + 1) * E] for ft in range(4)]

        mu = statp.tile([1, S], f32, tag="mu")
        s2 = statp.tile([1, S], f32, tag="s2")
        tmp = statp.tile([1, S], f32, tag="tmp")
        rstd = statp.tile([1, S], f32r, tag="rstd")
        betaB = statp.tile([1, S], bf16, tag="betaB")
        for h in range(2):
            cs = slice(h * 512, (h + 1) * 512)
            p1 = pmm.tile([1, 512], f32, tag="mm", name=f"st1_{h}")
            for i in range(ET):
                nc.tensor.matmul(p1[:], ones128[:], x_sb[i][:, cs],
                                 start=(i == 0), stop=(i == ET - 1))
            nc.vector.tensor_scalar(out=mu[:, cs], in0=p1[:], scalar1=1.0 / E,
                                    scalar2=None, op0=ALU.mult)
            p2 = pmm.tile([1, 512], f32, tag="mm", name=f"st2_{h}")
            for i in range(ET):
                sq = sqp.tile([128, 512], bf16, tag="sq", name=f"sq_{h}_{i}",
                              bufs=4)
                if h == 0:
                    nc.vector.tensor_mul(sq[:], x_sb[i][:, cs], x_sb[i][:, cs])
                else:
                    nc.scalar.activation(out=sq[:], in_=x_sb[i][:, cs],
                                         func=AF.Square, scale=1.0)
                nc.tensor.matmul(p2[:], ones128[:], sq[:],
                                 start=(i == 0), stop=(i == ET - 1))
            nc.vector.tensor_scalar(out=s2[:, cs], in0=p2[:], scalar1=1.0 / E,
                                    scalar2=None, op0=ALU.mult)
        nc.vector.tensor_mul(tmp[:], mu[:], mu[:])
        nc.vector.tensor_sub(s2[:], s2[:], tmp[:])
        nc.scalar.activation(out=tmp[:], in_=s2[:], func=AF.Ln, bias=eps[:], scale=1.0)
        nc.scalar.activation(out=rstd[:], in_=tmp[:], func=AF.Exp, scale=-0.5)
        nc.vector.tensor_scalar(out=betaB[:], in0=mu[:], scalar1=-1.0,
                                scalar2=None, op0=ALU.mult)

        rstdB = bcp.tile([128, S], f32, tag="rstdB")
        for h in range(2):
            cs = slice(h * 512, (h + 1) * 512)
            pb = pmm.tile([128, 512], f32, tag="mm", name=f"bcr_{h}")
            nc.tensor.matmul(pb[:], ones1f[:], rstd[:, cs],
                             start=True, stop=True)
            nc.vector.tensor_copy(out=rstdB[:, cs], in_=pb[:])

        # lx = (x - mu) * rstd  (gain/bias folded into weights host-side);
        # wide psum tiles keep the chain off the narrow pmm pool
        lxbig = lxp.tile([128, ET * S], bf16, tag="lx", name="lx")
        lx = [lxbig[:, i * S:(i + 1) * S] for i in range(ET)]
        for i in range(ET):
            pl = psc.tile([128, S], f32, tag="sc", name=f"pl_{i}")
            for h in range(2):
                cs = slice(h * 512, (h + 1) * 512)
                nc.tensor.matmul(pl[:, cs], ident[:], x_sb[i][:, cs],
                                 start=True, stop=False, skip_group_check=True)
                nc.tensor.matmul(pl[:, cs], ones1b[:], betaB[:, cs],
                                 start=False, stop=True, skip_group_check=True)
            nc.vector.tensor_mul(lx[i][:], pl[:], rstdB[:])
        xp.release()

        # ---------- phase 2a: V for all 8 heads, token-major + ones columns ---
        v_sb = []
        for tt in range(ET):
            pv = pmm.tile([128, 512], f32, tag="mm", name=f"pv_{tt}")
            for kt in range(ET):
                nc.tensor.matmul(pv[:], lx[kt][:, tt * 128:(tt + 1) * 128],
                                 wv_t(kt), start=(kt == 0), stop=(kt == ET - 1))
            vt = vsp.tile([128, 8 * 65], bf16, tag=f"v{tt}", name=f"v_{tt}")
            nc.vector.tensor_copy(
                out=vt[:].rearrange("p (h d) -> p h d", h=8)[:, :, 0:64],
                in_=pv[:].rearrange("p (h d) -> p h d", h=8))
            nc.vector.memset(
                vt[:].rearrange("p (h d) -> p h d", h=8)[:, :, 64:65], 1.0)
            v_sb.append(vt)

        # ---------- phase 2b: per head pair: Q/K -> scores -> softmax -> AV ---
        oT = []      # normalized attention outputs per pair [128, S] bf16

        for p in range(NP):
            qT = qkvp.tile([128, S], bf16, tag="qT", name=f"qT_{p}")
            kT = qkvp.tile([128, S], bf16, tag="kT", name=f"kT_{p}")
            for h in range(2):
                cs = slice(h * 512, (h + 1) * 512)
                pq = pmm.tile([128, 512], f32, tag="mm", name=f"pq_{p}_{h}")
                for kt in range(ET):
                    nc.tensor.matmul(pq[:], wq_t(p, kt), lx[kt][:, cs],
                                     start=(kt == 0), stop=(kt == ET - 1))
                nc.vector.tensor_scalar(out=qT[:, cs], in0=pq[:],
                                        scalar1=bqkv_sb[:, p:p + 1],
                                        scalar2=None, op0=ALU.add)
                pk = pmm.tile([128, 512], f32, tag="mm", name=f"pk_{p}_{h}")
                for kt in range(ET):
                    nc.tensor.matmul(pk[:], wk_t(p, kt), lx[kt][:, cs],
                                     start=(kt == 0), stop=(kt == ET - 1))
                nc.vector.tensor_scalar(out=kT[:, cs], in0=pk[:],
                                        scalar1=bqkv_sb[:, 4 + p:5 + p],
                                        scalar2=None, op0=ALU.add)

            # scores + exp per ktok tile; h0 rows 0-63 / h1 rows 64-127 of the
            # PE array run row-tiled concurrently (base partitions 0 / 64)
            at = [[None] * ET, [None] * ET]
            for tt in range(ET):
                ps_h = [psc.tile([128, S], f32, tag="sc", name=f"sc_{p}_{tt}_{h}")
                        for h in range(2)]
                for qc in range(2):
                    for h in range(2):
                        hsub = slice(h * 64, h * 64 + 64)
                        nc.tensor.matmul(ps_h[h][:, qc * 512:(qc + 1) * 512],
                                         kT[hsub, tt * 128:(tt + 1) * 128],
                                         qT[hsub, qc * 512:(qc + 1) * 512],
                                         start=True, stop=True,
                                         skip_group_check=True)
                for h in range(2):
                    a = attnp.tile([128, S], bf16, tag="attn",
                                   name=f"at_{p}_{tt}_{h}")
                    nc.scalar.activation(out=a[:], in_=ps_h[h][:], func=AF.Exp,
                                         scale=0.125)
                    at[h][tt] = a

            # AV + denominator (ones column), then per-pair normalize
            oTp = otp.tile([128, S], f32, tag=f"oT{p}", name=f"oT_{p}")
            recp = [statp.tile([1, S], f32r, tag=f"recp{h}", name=f"recp_{p}_{h}",
                               bufs=2) for h in range(2)]
            for h in range(2):
                hg = 2 * p + h
                hsub = slice(h * 64, h * 64 + 64)
                den_h = statp.tile([1, S], f32, tag=f"den{h}", name=f"den_{p}_{h}",
                                   bufs=2)
                po = [pav.tile([65, 512], f32, tag="av", name=f"pav_{p}_{h}_{qc}")
                      for qc in range(2)]
                for tt in range(ET):
                    vh = v_sb[tt][:].rearrange("p (h d) -> p h d", h=8)[:, hg, :]
                    for qc in range(2):
                        cs = slice(qc * 512, (qc + 1) * 512)
                        nc.tensor.matmul(po[qc][:], vh, at[h][tt][:, cs],
                                         start=(tt == 0), stop=(tt == ET - 1))
                for qc in range(2):
                    cs = slice(qc * 512, (qc + 1) * 512)
                    nc.vector.tensor_copy(out=den_h[:, cs], in_=po[qc][64:65, :])
                for qc in range(2):
                    cs = slice(qc * 512, (qc + 1) * 512)
                    nc.vector.tensor_copy(out=oTp[hsub, cs], in_=po[qc][0:64, :])
                # 1/den = exp(-ln(den)) on ACT (same table set as softmax exp)
                lnd = statp.tile([1, S], f32, tag=f"lnd{h}", name=f"lnd_{p}_{h}",
                                 bufs=2)
                nc.scalar.activation(out=lnd[:], in_=den_h[:], func=AF.Ln,
                                     scale=1.0)
                nc.scalar.activation(out=recp[h][:], in_=lnd[:], func=AF.Exp,
                                     scale=-1.0)

            ot_bf = otp.tile([128, S], bf16, tag=f"ob{p}", name=f"ob_{p}")
            for qc in range(2):
                cs = slice(qc * 512, (qc + 1) * 512)
                pr = pav.tile([128, 512], f32, tag="av", name=f"pr_{p}_{qc}")
                nc.tensor.matmul(pr[:], sel_h[0][:], recp[0][:, cs],
                                 start=True, stop=False)
                nc.tensor.matmul(pr[:], sel_h[1][:], recp[1][:, cs],
                                 start=False, stop=True)
                nc.vector.tensor_mul(ot_bf[:, cs], oTp[:, cs], pr[:])
            oT.append(ot_bf)

        # ---------- phase 3: partial out projection ----------
        for et in range(ET):
            for qc in range(2):
                cs = slice(qc * 512, (qc + 1) * 512)
                po = pmm.tile([128, 512], f32, tag="mm", name=f"po_{et}_{qc}")
                for ft in range(4):
                    nc.tensor.matmul(po[:], wo_t[ft][:, et * 128:(et + 1) * 128],
                                     oT[ft][:, cs],
                                     start=(ft == 0), stop=(ft == 3))
                ot = outp.tile([128, 512], bf16, tag="out", name=f"o_{et}_{qc}")
                if et % 2 == 0:
                    nc.vector.tensor_copy(out=ot[:], in_=po[:])
                else:
                    nc.scalar.activation(out=ot[:], in_=po[:], func=AF.Identity,
                                         scale=1.0)
                eng = nc.sync if (et + qc) % 2 == 0 else nc.gpsimd
                eng.dma_start(out=outT_d[et * 128:(et + 1) * 128, cs], in_=ot[:])

        lxp.release()
        outp.release()
        otp.release()
        attnp.release()
        vsp.release()
        qkvp.release()
        wsp.release()
        sqp.release()
        bcp.release()
        statp.release()
        consts.release()
        pav.release()
        psc.release()
        pmm.release()

    nc.compile()
    return nc


def _build_launch2():
    nc = bacc.Bacc("TRN2", target_bir_lowering=False, debug=False, num_devices=NCORES)

    toksT_d = nc.dram_tensor("toksT", [E, C], bf16, kind="ExternalInput").ap()
    w1_d = nc.dram_tensor("w1", [E, F], bf16, kind="ExternalInput").ap()
    w2_d = nc.dram_tensor("w2", [F, E], bf16, kind="ExternalInput").ap()
    b1_d = nc.dram_tensor("b1", [F, 1], f32, kind="ExternalInput").ap()
    b2_d = nc.dram_tensor("b2", [E, 1], f32, kind="ExternalInput").ap()
    outT_d = nc.dram_tensor("outT", [E, C], bf16, kind="ExternalOutput").ap()

    with tile.TileContext(nc) as tc:
        with (
            tc.tile_pool(name="consts", bufs=1) as consts,
            tc.tile_pool(name="tok", bufs=1) as tokp,
            tc.tile_pool(name="hp", bufs=1) as hp,
            tc.tile_pool(name="ws", bufs=6) as wsp,
            tc.tile_pool(name="outs", bufs=3) as outs,
            tc.tile_pool(name="pg1", bufs=4, space="PSUM") as pg1,
            tc.tile_pool(name="pg2", bufs=4, space="PSUM") as pg2,
        ):
            dust = consts.tile([1, 1], f32, tag="dust")
            nc.vector.memset(dust[:], 0.25)
            nc.scalar.activation(out=dust[:], in_=dust[:], func=_GELU, scale=1.0)

            tokbig = tokp.tile([128, ET * C], bf16, tag="t", name="toks")
            tok_r = tokbig[:].rearrange("p (a c) -> p a c", a=ET)
            tokd_r = toksT_d.rearrange("(a p) c -> p a c", p=128)
            for ci, (off, w) in enumerate(CT):
                eng = (nc.sync, nc.scalar, nc.gpsimd)[ci % 3]
                eng.dma_start(out=tok_r[:, :, off:off + w],
                              in_=tokd_r[:, :, off:off + w])
            toks = [tokbig[:, i * C:(i + 1) * C] for i in range(ET)]

            b1_sb = consts.tile([128, FT], f32, tag="b1")
            nc.gpsimd.dma_start(out=b1_sb[:],
                                in_=b1_d.rearrange("(a p) o -> p (a o)", p=128))
            b2_sb = consts.tile([128, ET], f32, tag="b2")
            nc.gpsimd.dma_start(out=b2_sb[:],
                                in_=b2_d.rearrange("(a p) o -> p (a o)", p=128))

            hbf = []
            for ft in range(FT):
                hbf.append(hp.tile([128, C], bf16, tag=f"h{ft}", name=f"hbf{ft}"))

            # GEMM1: hT = gelu(w1.T @ toksT + b1)
            # weight DMAs pull [2 kt x 512 cols] per transfer (four ft tiles)
            w1_r = w1_d.rearrange("(a p) c -> p a c", p=128)
            for ftp in range(FT // 4):
                blks = []
                for kj in range(ET // 2):
                    wt = wsp.tile([128, 2 * 512], bf16, tag="w1",
                                  name=f"w1_{ftp}_{kj}", bufs=10)
                    eng = (nc.sync, nc.gpsimd, nc.scalar)[kj % 3]
                    eng.dma_start(
                        out=wt[:].rearrange("p (a c) -> p a c", a=2),
                        in_=w1_r[:, 2 * kj:2 * kj + 2,
                                 ftp * 512:(ftp + 1) * 512])
                    blks.append(wt)
                for sub in range(4):
                    ft = ftp * 4 + sub
                    ps = [pg1.tile([128, w], f32, tag="g1", name=f"pg1_{ft}_{ci}")
                          for ci, (off, w) in enumerate(CT)]
                    for kt in range(ET):
                        wv = blks[kt // 2][:, (kt % 2) * 512 + sub * 128:
                                           (kt % 2) * 512 + (sub + 1) * 128]
                        for ci, (off, w) in enumerate(CT):
                            nc.tensor.matmul(ps[ci][:], wv,
                                             toks[kt][:, off:off + w],
                                             start=(kt == 0), stop=(kt == ET - 1))
                    for ci, (off, w) in enumerate(CT):
                        nc.scalar.activation(out=hbf[ft][:, off:off + w], in_=ps[ci][:],
                                             func=_GELU, bias=b1_sb[:, ft:ft + 1],
                                             scale=1.0)

            # GEMM2: outT = w2.T @ hT + b2
            # weight blocks [128, 512] cover four et tiles, kept resident across
            # the et accumulations
            w2_r = w2_d.rearrange("(a p) c -> p a c", p=128)
            for etp in range(ET // 4):
                blks = []
                for fj in range(FT // 2):
                    wt = wsp.tile([128, 2 * 512], bf16, tag="w2",
                                  name=f"w2_{etp}_{fj}", bufs=20)
                    eng = (nc.sync, nc.gpsimd, nc.scalar)[fj % 3]
                    eng.dma_start(
                        out=wt[:].rearrange("p (a c) -> p a c", a=2),
                        in_=w2_r[:, 2 * fj:2 * fj + 2,
                                 etp * 512:(etp + 1) * 512])
                    blks.append(wt)
                for sub in range(4):
                    et = etp * 4 + sub
                    ps = [pg2.tile([128, w], f32, tag="g2", name=f"pg2_{et}_{ci}")
                          for ci, (off, w) in enumerate(CT)]
                    for ft in range(FT):
                        wv = blks[ft // 2][:, (ft % 2) * 512 + sub * 128:
                                           (ft % 2) * 512 + (sub + 1) * 128]
                        for ci, (off, w) in enumerate(CT):
                            nc.tensor.matmul(ps[ci][:], wv, hbf[ft][:, off:off + w],
                                             start=(ft == 0), stop=(ft == FT - 1))
                    for ci, (off, w) in enumerate(CT):
                        ot = outs.tile([128, 512], bf16, tag="ot", name=f"ot_{et}_{ci}")
                        nc.vector.tensor_scalar(out=ot[:, 0:w], in0=ps[ci][:],
                                                scalar1=b2_sb[:, et:et + 1],
                                                scalar2=None, op0=ALU.add)
                        eng = nc.sync if (et + ci) % 2 == 0 else nc.gpsimd
                        eng.dma_start(
                            out=outT_d[et * 128:(et + 1) * 128, off:off + w],
                            in_=ot[:, 0:w])

    nc.compile()
    return nc


def _get_programs():
    if "l1" not in _programs:
        _programs["l1"] = _build_launch1()
    if "l2" not in _programs:
        _programs["l2"] = _build_launch2()
    return _programs["l1"], _programs["l2"]


def _expert_ffn_host(toks, w1e, b1e, w2e, b2e):
    """Exact host fallback for capacity overflow (rare)."""
    from scipy.special import erf
    h = toks @ w1e + b1e
    h = 0.5 * h * (1.0 + erf(h / np.float32(np.sqrt(2.0))))
    return h.astype(np.float32) @ w2e + b2e


def kernel(**inputs):
    import ml_dtypes

    l1, l2 = _get_programs()

    x = np.ascontiguousarray(np.asarray(inputs["x"], dtype=np.float32))        # (S,B,E)
    in_w = np.asarray(inputs["in_proj_w"], dtype=np.float32)                   # (3E,E)
    in_b = np.asarray(inputs["in_proj_b"], dtype=np.float32)
    out_w = np.asarray(inputs["out_proj_w"], dtype=np.float32)                 # (E,E)
    out_b = np.asarray(inputs["out_proj_b"], dtype=np.float32)
    gate_w = np.asarray(inputs["gate_w"], dtype=np.float32)                    # (NE,E)
    w1 = np.asarray(inputs["w1"], dtype=np.float32)                            # (NE,E,F)
    b1 = np.asarray(inputs["b1"], dtype=np.float32)
    w2 = np.asarray(inputs["w2"], dtype=np.float32)                            # (NE,F,E)
    b2 = np.asarray(inputs["b2"], dtype=np.float32)
    ln1_g = np.asarray(inputs["ln1_g"], dtype=np.float32)
    ln1_b = np.asarray(inputs["ln1_b"], dtype=np.float32)
    ln2_g = np.asarray(inputs["ln2_g"], dtype=np.float32)
    ln2_b = np.asarray(inputs["ln2_b"], dtype=np.float32)

    bf = ml_dtypes.bfloat16
    col = lambda v: np.ascontiguousarray(v.reshape(-1, 1))

    # fold LN1 gain into the QKV weights; LN1 bias into the QKV bias
    in_w_f = in_w * ln1_g[None, :]                 # (3E, E)
    in_b_f = in_b + in_w @ ln1_b                   # (3E,)

    # per-core head-group weight packs: pair-major [q 128 | k 128 | v 128]
    # head-group g covers heads 8g..8g+7 -> feature rows 512g..512(g+1)
    wqkv_g, bqkv_g, woT_g = [], [], []
    for g in range(2):
        fs = slice(512 * g, 512 * (g + 1))
        wq = in_w_f[0 * E:1 * E][fs]               # (512, E)
        wk = in_w_f[1 * E:2 * E][fs]
        wv = in_w_f[2 * E:3 * E][fs]
        pack = np.concatenate([wq.T, wk.T, wv.T], axis=1)   # (E, 1536)
        bias = np.zeros((8 * 128, 1), dtype=np.float32)
        bias[0:512, 0] = in_b_f[0 * E:1 * E][fs]
        bias[512:1024, 0] = in_b_f[1 * E:2 * E][fs]
        wqkv_g.append(np.ascontiguousarray(pack.astype(bf)))
        bqkv_g.append(bias)
        woT_g.append(np.ascontiguousarray(out_w[:, fs].T.astype(bf)))  # (512, E)

    sel2 = np.zeros((2, 128), dtype=np.float32)
    sel2[0, 0:64] = 1.0
    sel2[1, 64:128] = 1.0
    ident = np.eye(128, dtype=np.float32).astype(bf)

    # v-bias folds into a constant added on the host:
    # (o/den + bv_g) @ Wo_g.T summed over g  ->  + bv @ Wo.T (+ out_b)
    bv = in_b_f[2 * E:3 * E]
    const_out = out_b + bv @ out_w.T               # (E,)

    # ---- launch 1 ----
    xT_b = [np.ascontiguousarray(x[:, b, :].T.astype(bf)) for b in range(B)]
    in_maps1 = []
    for c in range(NCORES):
        b, g = divmod(c, 2)
        in_maps1.append({
            "xT": xT_b[b],
            "wqkv": wqkv_g[g],
            "bqkv": bqkv_g[g],
            "woT": woT_g[g],
            "sel2": sel2,
            "ident": ident,
        })
    res1 = run_bass_kernel_spmd(l1, in_maps1, list(range(NCORES)))

    # combine partials + residual -> x2 [E, S, B]; then LN2 + gating on host
    x2_all = np.empty((E, S, B), dtype=np.float32)
    for b in range(B):
        x2_all[:, :, b] = (x[:, b, :].T
                           + res1.results[2 * b]["outT"].astype(np.float32)
                           + res1.results[2 * b + 1]["outT"].astype(np.float32)
                           + const_out[:, None])
    x2_flat = x2_all.reshape(E, N)                 # token n = s*B + b

    mu = x2_flat.mean(axis=0)
    var = x2_flat.var(axis=0)
    h2_flat = ((x2_flat - mu) / np.sqrt(var + 1e-5)) * ln2_g[:, None] + ln2_b[:, None]
    h2_flat = h2_flat.astype(np.float32)

    # ---- host gating: softmax over NE logits, top-2 renormalized ----
    logits = gate_w @ h2_flat                      # (NE, N)
    logits -= logits.max(axis=0, keepdims=True)
    p = np.exp(logits)
    p /= p.sum(axis=0, keepdims=True)
    ar = np.arange(N)
    i1 = np.argmax(p, axis=0)
    v1 = p[i1, ar]
    pm = p.copy()
    pm[i1, ar] = -1.0
    i2 = np.argmax(pm, axis=0)
    v2 = p[i2, ar]
    gsum = v1 + v2
    gate1 = v1 / gsum
    gate2 = v2 / gsum

    h2_bf = h2_flat.astype(bf)
    idx_list, gates_list, ov_list = [], [], []
    in_maps2 = []
    for e in range(NE):
        sel = np.where((i1 == e) | (i2 == e))[0]
        ge = np.where(i1[sel] == e, gate1[sel], gate2[sel]).astype(np.float32)
        ov = None
        if len(sel) > C:
            ov = (sel[C:], ge[C:])
            sel, ge = sel[:C], ge[:C]
        idx_list.append(sel)
        gates_list.append(ge)
        ov_list.append(ov)
        toksT = np.zeros((E, C), dtype=bf)
        toksT[:, :len(sel)] = h2_bf[:, sel]
        in_maps2.append({
            "toksT": toksT,
            "w1": w1[e].astype(bf),
            "w2": w2[e].astype(bf),
            "b1": col(b1[e]),
            "b2": col(b2[e]),
        })
    res2 = run_bass_kernel_spmd(l2, in_maps2, list(range(NCORES)))

    # ---- combine ----
    out_flat = x2_flat
    for e in range(NE):
        sel, ge = idx_list[e], gates_list[e]
        out_flat[:, sel] += (res2.results[e]["outT"][:, :len(sel)].astype(np.float32)
                             * ge[None, :])
        if ov_list[e] is not None:
            osel, oge = ov_list[e]
            oo = _expert_ffn_host(h2_flat[:, osel].T, w1[e], b1[e], w2[e], b2[e])
            out_flat[:, osel] += oo.T * oge[None, :]

    return np.ascontiguousarray(
        out_flat.reshape(E, S, B).transpose(1, 2, 0)).astype(np.float32)


# revision 34
# speedup vs baseline: 1.0201x; 1.0201x over previous
"""MoE transformer layer on 8 Trainium2 NeuronCores.

Strategy:
  Launch 1 (attention): shard by (batch, head-group) -> 8 cores.
    Core (b, g) holds all 1024 tokens of batch b and computes LN1 ->
    Q/K/V for its 8 heads -> softmax -> AV -> its partial of the output
    projection, all in bf16 with features on partitions. No K/V
    duplication across cores; LN1 gain/bias are folded into the QKV
    weights on the host. Output: partial attn projection [E, S] bf16.
  Host: combine the two partials per batch + residual -> x2; LN2 ->
    h2; top-2 gating (softmax over 8 logits, renormalized); builds the
    per-expert token batches (all-to-all dispatch done on host).
  Launch 2 (expert FFN): expert-parallel, core e owns expert e.
    toksT [E, C] bf16 -> gelu(w1.T @ toks + b1) -> w2.T @ h + b2.
  Host: scatter-add combine with gate weights + residual.
"""

import numpy as np

import concourse.bass as bass
import concourse.tile as tile
from concourse import bacc, mybir
from concourse.bass_utils import run_bass_kernel_spmd

S, B, E = 1024, 4, 1024
H, DH = 16, 64
F, NE = 4096, 8
N = S * B
NCORES = 8
C = 1088         # expert capacity (max expert load for seed-0 inputs is ~1076)
CT = [(0, 512), (512, 512), (1024, 64)]  # (offset, width) token tiles in launch 2
ET = E // 128    # 8
FT = F // 128    # 32
NP = 4           # head pairs per core (8 heads)

f32 = mybir.dt.float32
f32r = mybir.dt.float32r
bf16 = mybir.dt.bfloat16
AF = mybir.ActivationFunctionType
ALU = mybir.AluOpType

_GELU = AF.Gelu

_programs = {}


def _build_launch1():
    nc = bacc.Bacc("TRN2", target_bir_lowering=False, debug=False, num_devices=NCORES)

    xT_d = nc.dram_tensor("xT", [E, S], bf16, kind="ExternalInput").ap()
    wqkv_d = nc.dram_tensor("wqkv", [E, 1536], bf16, kind="ExternalInput").ap()
    bqkv_d = nc.dram_tensor("bqkv", [8 * 128, 1], f32, kind="ExternalInput").ap()
    woT_d = nc.dram_tensor("woT", [512, E], bf16, kind="ExternalInput").ap()
    sel2_d = nc.dram_tensor("sel2", [2, 128], f32, kind="ExternalInput").ap()
    ident_d = nc.dram_tensor("ident", [128, 128], bf16, kind="ExternalInput").ap()
    outT_d = nc.dram_tensor("outT", [E, S], bf16, kind="ExternalOutput").ap()

    with tile.TileContext(nc) as tc:
        consts = tc.alloc_tile_pool(name="consts", bufs=1)
        statp = tc.alloc_tile_pool(name="stat", bufs=1)
        bcp = tc.alloc_tile_pool(name="bc", bufs=1)
        sqp = tc.alloc_tile_pool(name="sqp", bufs=2)
        wsp = tc.alloc_tile_pool(name="wstream", bufs=1)
        qkvp = tc.alloc_tile_pool(name="qkvp", bufs=2)
        vsp = tc.alloc_tile_pool(name="vsp", bufs=1)
        attnp = tc.alloc_tile_pool(name="attnp", bufs=12)
        otp = tc.alloc_tile_pool(name="otp", bufs=1)
        outp = tc.alloc_tile_pool(name="outp", bufs=3)
        pmm = tc.alloc_tile_pool(name="pmm", bufs=2, space="PSUM")
        psc = tc.alloc_tile_pool(name="psc", bufs=2, space="PSUM")
        pav = tc.alloc_tile_pool(name="pav", bufs=2, space="PSUM")

        ones128 = consts.tile([128, 1], bf16, tag="ones128")
        nc.vector.memset(ones128[:], 1.0)
        ones1b = consts.tile([1, 128], bf16, tag="ones1b")
        nc.vector.memset(ones1b[:], 1.0)
        ones1f = consts.tile([1, 128], f32r, tag="ones1f")
        nc.vector.memset(ones1f[:].bitcast(f32), 1.0)
        eps = consts.tile([1, 1], f32, tag="eps")
        nc.vector.memset(eps[:], 1e-5)
        dust = consts.tile([1, 1], f32, tag="dust")

        # prime the ACT Ln/Exp table set right away
        nc.scalar.activation(out=dust[:], in_=eps[:], func=AF.Ln, scale=1.0)
        nc.scalar.activation(out=dust[:], in_=dust[:], func=AF.Exp, scale=1.0)

        ident = consts.tile([128, 128], bf16, tag="ident")
        nc.gpsimd.dma_start(out=ident[:], in_=ident_d)
        sel_h = []
        for h in range(2):
            st = consts.tile([1, 128], f32, tag=f"sel{h}")
            nc.gpsimd.dma_start(out=st[:], in_=sel2_d[h:h + 1, :])
            sel_h.append(st)
        bqkv_sb = consts.tile([128, 8], f32, tag="bqkv")
        nc.gpsimd.dma_start(out=bqkv_sb[:],
                            in_=bqkv_d.rearrange("(a p) o -> p (a o)", p=128))

        # ---------- phase 1: load x (bf16, row-block split), LN1 stats -------
        lxp = tc.alloc_tile_pool(name="lxp", bufs=1)
        xp = tc.alloc_tile_pool(name="xp", bufs=1)

        xbig = xp.tile([128, ET * S], bf16, tag="x", name="x_sb")
        xT_r = xT_d.rearrange("(a p) c -> p a c", p=128)
        xbig_r = xbig[:].rearrange("p (a c) -> p a c", a=ET)
        nc.sync.dma_start(out=xbig_r[:, 0:4, :], in_=xT_r[:, 0:4, :])
        nc.scalar.dma_start(out=xbig_r[:, 4:8, :], in_=xT_r[:, 4:8, :])
        x_sb = [xbig[:, i * S:(i + 1) * S] for i in range(ET)]

        # all qkv weights resident; three engine-parallel row-block DMAs
        w_sb = wsp.tile([128, ET * 1536], bf16, tag="wqkv", name="w_sb")
        w_r = w_sb[:].rearrange("p (a c) -> p a c", a=ET)
        wd_r = wqkv_d.rearrange("(a p) c -> p a c", p=128)
        nc.gpsimd.dma_start(out=w_r[:, 0:3, :], in_=wd_r[:, 0:3, :])
        nc.sync.dma_start(out=w_r[:, 3:6, :], in_=wd_r[:, 3:6, :])
        nc.scalar.dma_start(out=w_r[:, 6:8, :], in_=wd_r[:, 6:8, :])

        def wq_t(p, kt):
            return w_sb[:, kt * 1536 + p * 128: kt * 1536 + (p + 1) * 128]

        def wk_t(p, kt):
            return w_sb[:, kt * 1536 + 512 + p * 128: kt * 1536 + 512 + (p + 1) * 128]

        def wv_t(kt):
            return w_sb[:, kt * 1536 + 1024: kt * 1536 + 1536]

        # out-proj weights: one early DMA
        wo_big = wsp.tile([128, 4 * E], bf16, tag="wo", name="wo")
        nc.gpsimd.dma_start(
            out=wo_big[:].rearrange("p (a c) -> p a c", a=4),
            in_=woT_d.rearrange("(a p) c -> p a c", p=128))
        wo_t = [wo_big[:, ft * E:(ft + 1) * E] for ft in range(4)]

        mu = statp.tile([1, S], f32, tag="mu")
        s2 = statp.tile([1, S], f32, tag="s2")
        tmp = statp.tile([1, S], f32, tag="tmp")
        rstd = statp.tile([1, S], f32r, tag="rstd")
        betaB = statp.tile([1, S], bf16, tag="betaB")
        for h in range(2):
            cs = slice(h * 512, (h + 1) * 512)
            p1 = pmm.tile([1, 512], f32, tag="mm", name=f"st1_{h}")
            for i in range(ET):
                nc.tensor.matmul(p1[:], ones128[:], x_sb[i][:, cs],
                                 start=(i == 0), stop=(i == ET - 1))
            nc.vector.tensor_scalar(out=mu[:, cs], in0=p1[:], scalar1=1.0 / E,
                                    scalar2=None, op0=ALU.mult)
            p2 = pmm.tile([1, 512], f32, tag="mm", name=f"st2_{h}")
            for i in range(ET):
                sq = sqp.tile([128, 512], bf16, tag="sq", name=f"sq_{h}_{i}",
                              bufs=4)
                nc.vector.tensor_mul(sq[:], x_sb[i][:, cs], x_sb[i][:, cs])
                nc.tensor.matmul(p2[:], ones128[:], sq[:],
                                 start=(i == 0), stop=(i == ET - 1))
            nc.vector.tensor_scalar(out=s2[:, cs], in0=p2[:], scalar1=1.0 / E,
                                    scalar2=None, op0=ALU.mult)
        nc.vector.tensor_mul(tmp[:], mu[:], mu[:])
        nc.vector.tensor_sub(s2[:], s2[:], tmp[:])
        nc.scalar.activation(out=tmp[:], in_=s2[:], func=AF.Ln, bias=eps[:], scale=1.0)
        nc.scalar.activation(out=rstd[:], in_=tmp[:], func=AF.Exp, scale=-0.5)
        nc.vector.tensor_scalar(out=betaB[:], in0=mu[:], scalar1=-1.0,
                                scalar2=None, op0=ALU.mult)

        rstdB = bcp.tile([128, S], f32, tag="rstdB")
        for h in range(2):
            cs = slice(h * 512, (h + 1) * 512)
            pb = pmm.tile([128, 512], f32, tag="mm", name=f"bcr_{h}")
            nc.tensor.matmul(pb[:], ones1f[:], rstd[:, cs],
                             start=True, stop=True)
            nc.vector.tensor_copy(out=rstdB[:, cs], in_=pb[:])

        # lx = (x - mu) * rstd  (gain/bias folded into weights host-side);
        # wide psum tiles keep the chain off the narrow pmm pool
        lxbig = lxp.tile([128, ET * S], bf16, tag="lx", name="lx")
        lx = [lxbig[:, i * S:(i + 1) * S] for i in range(ET)]
        for i in range(ET):
            pl = psc.tile([128, S], f32, tag="sc", name=f"pl_{i}")
            for h in range(2):
                cs = slice(h * 512, (h + 1) * 512)
                nc.tensor.matmul(pl[:, cs], ident[:], x_sb[i][:, cs],
                                 start=True, stop=False, skip_group_check=True)
                nc.tensor.matmul(pl[:, cs], ones1b[:], betaB[:, cs],
                                 start=False, stop=True, skip_group_check=True)
            nc.vector.tensor_mul(lx[i][:], pl[:], rstdB[:])
        xp.release()

        # ---------- phase 2a: V for all 8 heads, token-major + ones columns ---
        v_sb = []
        for tt in range(ET):
            pv = pmm.tile([128, 512], f32, tag="mm", name=f"pv_{tt}")
            for kt in range(ET):
                nc.tensor.matmul(pv[:], lx[kt][:, tt * 128:(tt + 1) * 128],
                                 wv_t(kt), start=(kt == 0), stop=(kt == ET - 1))
            vt = vsp.tile([128, 8 * 65], bf16, tag=f"v{tt}", name=f"v_{tt}")
            nc.vector.tensor_copy(
                out=vt[:].rearrange("p (h d) -> p h d", h=8)[:, :, 0:64],
                in_=pv[:].rearrange("p (h d) -> p h d", h=8))
            nc.vector.memset(
                vt[:].rearrange("p (h d) -> p h d", h=8)[:, :, 64:65], 1.0)
            v_sb.append(vt)

        # ---------- phase 2b: per head pair: Q/K -> scores -> softmax -> AV ---
        oT = []      # normalized attention outputs per pair [128, S] bf16

        for p in range(NP):
            qT = qkvp.tile([128, S], bf16, tag="qT", name=f"qT_{p}")
            kT = qkvp.tile([128, S], bf16, tag="kT", name=f"kT_{p}")
            for h in range(2):
                cs = slice(h * 512, (h + 1) * 512)
                pq = pmm.tile([128, 512], f32, tag="mm", name=f"pq_{p}_{h}")
                for kt in range(ET):
                    nc.tensor.matmul(pq[:], wq_t(p, kt), lx[kt][:, cs],
                                     start=(kt == 0), stop=(kt == ET - 1))
                nc.vector.tensor_scalar(out=qT[:, cs], in0=pq[:],
                                        scalar1=bqkv_sb[:, p:p + 1],
                                        scalar2=None, op0=ALU.add)
                pk = pmm.tile([128, 512], f32, tag="mm", name=f"pk_{p}_{h}")
                for kt in range(ET):
                    nc.tensor.matmul(pk[:], wk_t(p, kt), lx[kt][:, cs],
                                     start=(kt == 0), stop=(kt == ET - 1))
                nc.vector.tensor_scalar(out=kT[:, cs], in0=pk[:],
                                        scalar1=bqkv_sb[:, 4 + p:5 + p],
                                        scalar2=None, op0=ALU.add)

            # scores + exp per ktok tile; h0 rows 0-63 / h1 rows 64-127 of the
            # PE array run row-tiled concurrently (base partitions 0 / 64)
            at = [[None] * ET, [None] * ET]
            for tt in range(ET):
                ps_h = [psc.tile([128, S], f32, tag="sc", name=f"sc_{p}_{tt}_{h}")
                        for h in range(2)]
                for qc in range(2):
                    for h in range(2):
                        hsub = slice(h * 64, h * 64 + 64)
                        nc.tensor.matmul(ps_h[h][:, qc * 512:(qc + 1) * 512],
                                         kT[hsub, tt * 128:(tt + 1) * 128],
                                         qT[hsub, qc * 512:(qc + 1) * 512],
                                         start=True, stop=True,
                                         skip_group_check=True)
                for h in range(2):
                    a = attnp.tile([128, S], bf16, tag="attn",
                                   name=f"at_{p}_{tt}_{h}")
                    nc.scalar.activation(out=a[:], in_=ps_h[h][:], func=AF.Exp,
                                         scale=0.125)
                    at[h][tt] = a

            # AV + denominator (ones column), then per-pair normalize
            oTp = otp.tile([128, S], f32, tag=f"oT{p}", name=f"oT_{p}")
            recp = [statp.tile([1, S], f32, tag=f"recp{h}", name=f"recp_{p}_{h}",
                               bufs=2) for h in range(2)]
            for h in range(2):
                hg = 2 * p + h
                hsub = slice(h * 64, h * 64 + 64)
                den_h = statp.tile([1, S], f32, tag=f"den{h}", name=f"den_{p}_{h}",
                                   bufs=2)
                po = [pav.tile([65, 512], f32, tag="av", name=f"pav_{p}_{h}_{qc}")
                      for qc in range(2)]
                for tt in range(ET):
                    vh = v_sb[tt][:].rearrange("p (h d) -> p h d", h=8)[:, hg, :]
                    for qc in range(2):
                        cs = slice(qc * 512, (qc + 1) * 512)
                        nc.tensor.matmul(po[qc][:], vh, at[h][tt][:, cs],
                                         start=(tt == 0), stop=(tt == ET - 1))
                for qc in range(2):
                    cs = slice(qc * 512, (qc + 1) * 512)
                    nc.vector.tensor_copy(out=den_h[:, cs], in_=po[qc][64:65, :])
                for qc in range(2):
                    cs = slice(qc * 512, (qc + 1) * 512)
                    nc.vector.tensor_copy(out=oTp[hsub, cs], in_=po[qc][0:64, :])
                nc.vector.reciprocal_approx_fast(out=recp[h][:], in_=den_h[:])

            ot_bf = otp.tile([128, S], bf16, tag=f"ob{p}", name=f"ob_{p}")
            for qc in range(2):
                cs = slice(qc * 512, (qc + 1) * 512)
                pr = pav.tile([128, 512], f32, tag="av", name=f"pr_{p}_{qc}")
                nc.tensor.matmul(pr[:], sel_h[0][:], recp[0][:, cs],
                                 start=True, stop=False)
                nc.tensor.matmul(pr[:], sel_h[1][:], recp[1][:, cs],
                                 start=False, stop=True)
                nc.vector.tensor_mul(ot_bf[:, cs], oTp[:, cs], pr[:])
            oT.append(ot_bf)

        # ---------- phase 3: partial out projection ----------
        for et in range(ET):
            ot = outp.tile([128, S], bf16, tag="out", name=f"o_{et}")
            for qc in range(2):
                cs = slice(qc * 512, (qc + 1) * 512)
                po = pmm.tile([128, 512], f32, tag="mm", name=f"po_{et}_{qc}")
                for ft in range(4):
                    nc.tensor.matmul(po[:], wo_t[ft][:, et * 128:(et + 1) * 128],
                                     oT[ft][:, cs],
                                     start=(ft == 0), stop=(ft == 3))
                if et % 2 == 0:
                    nc.vector.tensor_copy(out=ot[:, cs], in_=po[:])
                else:
                    nc.scalar.activation(out=ot[:, cs], in_=po[:],
                                         func=AF.Identity, scale=1.0)
            eng = nc.sync if et % 2 == 0 else nc.gpsimd
            eng.dma_start(out=outT_d[et * 128:(et + 1) * 128, :], in_=ot[:])

        lxp.release()
        outp.release()
        otp.release()
        attnp.release()
        vsp.release()
        qkvp.release()
        wsp.release()
        sqp.release()
        bcp.release()
        statp.release()
        consts.release()
        pav.release()
        psc.release()
        pmm.release()

    nc.compile()
    return nc


def _build_launch2():
    nc = bacc.Bacc("TRN2", target_bir_lowering=False, debug=False, num_devices=NCORES)

    toksT_d = nc.dram_tensor("toksT", [E, C], bf16, kind="ExternalInput").ap()
    w1_d = nc.dram_tensor("w1", [E, F], bf16, kind="ExternalInput").ap()
    w2_d = nc.dram_tensor("w2", [F, E], bf16, kind="ExternalInput").ap()
    b1_d = nc.dram_tensor("b1", [F, 1], f32, kind="ExternalInput").ap()
    b2_d = nc.dram_tensor("b2", [E, 1], f32, kind="ExternalInput").ap()
    outT_d = nc.dram_tensor("outT", [E, C], bf16, kind="ExternalOutput").ap()

    with tile.TileContext(nc) as tc:
        with (
            tc.tile_pool(name="consts", bufs=1) as consts,
            tc.tile_pool(name="tok", bufs=1) as tokp,
            tc.tile_pool(name="hp", bufs=1) as hp,
            tc.tile_pool(name="ws", bufs=6) as wsp,
            tc.tile_pool(name="outs", bufs=3) as outs,
            tc.tile_pool(name="pg1", bufs=4, space="PSUM") as pg1,
            tc.tile_pool(name="pg2", bufs=4, space="PSUM") as pg2,
        ):
            dust = consts.tile([1, 1], f32, tag="dust")
            nc.vector.memset(dust[:], 0.25)
            nc.scalar.activation(out=dust[:], in_=dust[:], func=_GELU, scale=1.0)

            tokbig = tokp.tile([128, ET * C], bf16, tag="t", name="toks")
            tok_r = tokbig[:].rearrange("p (a c) -> p a c", a=ET)
            tokd_r = toksT_d.rearrange("(a p) c -> p a c", p=128)
            nc.sync.dma_start(out=tok_r[:, 0:4, :], in_=tokd_r[:, 0:4, :])
            nc.scalar.dma_start(out=tok_r[:, 4:8, :], in_=tokd_r[:, 4:8, :])
            toks = [tokbig[:, i * C:(i + 1) * C] for i in range(ET)]

            b1_sb = consts.tile([128, FT], f32, tag="b1")
            nc.gpsimd.dma_start(out=b1_sb[:],
                                in_=b1_d.rearrange("(a p) o -> p (a o)", p=128))
            b2_sb = consts.tile([128, ET], f32, tag="b2")
            nc.gpsimd.dma_start(out=b2_sb[:],
                                in_=b2_d.rearrange("(a p) o -> p (a o)", p=128))

            hbf = []
            for ft in range(FT):
                hbf.append(hp.tile([128, C], bf16, tag=f"h{ft}", name=f"hbf{ft}"))

            # GEMM1: hT = gelu(w1.T @ toksT + b1)
            # weight DMAs pull [2 kt x 512 cols] per transfer (four ft tiles)
            w1_r = w1_d.rearrange("(a p) c -> p a c", p=128)
            for ftp in range(FT // 4):
                blks = []
                for kj in range(ET // 2):
                    wt = wsp.tile([128, 2 * 512], bf16, tag="w1",
                                  name=f"w1_{ftp}_{kj}", bufs=10)
                    eng = (nc.sync, nc.gpsimd, nc.scalar)[kj % 3]
                    eng.dma_start(
                        out=wt[:].rearrange("p (a c) -> p a c", a=2),
                        in_=w1_r[:, 2 * kj:2 * kj + 2,
                                 ftp * 512:(ftp + 1) * 512])
                    blks.append(wt)
                for sub in range(4):
                    ft = ftp * 4 + sub
                    ps = [pg1.tile([128, w], f32, tag="g1", name=f"pg1_{ft}_{ci}")
                          for ci, (off, w) in enumerate(CT)]
                    for kt in range(ET):
                        wv = blks[kt // 2][:, (kt % 2) * 512 + sub * 128:
                                           (kt % 2) * 512 + (sub + 1) * 128]
                        for ci, (off, w) in enumerate(CT):
                            nc.tensor.matmul(ps[ci][:], wv,
                                             toks[kt][:, off:off + w],
                                             start=(kt == 0), stop=(kt == ET - 1))
                    for ci, (off, w) in enumerate(CT):
                        nc.scalar.activation(out=hbf[ft][:, off:off + w], in_=ps[ci][:],
                                             func=_GELU, bias=b1_sb[:, ft:ft + 1],
                                             scale=1.0)

            # GEMM2: outT = w2.T @ hT + b2
            # weight blocks [128, 512] cover four et tiles, kept resident across
            # the et accumulations
            w2_r = w2_d.rearrange("(a p) c -> p a c", p=128)
            for etp in range(ET // 4):
                blks = []
                for fj in range(FT // 2):
                    wt = wsp.tile([128, 2 * 512], bf16, tag="w2",
                                  name=f"w2_{etp}_{fj}", bufs=20)
                    eng = (nc.sync, nc.gpsimd, nc.scalar)[fj % 3]
                    eng.dma_start(
                        out=wt[:].rearrange("p (a c) -> p a c", a=2),
                        in_=w2_r[:, 2 * fj:2 * fj + 2,
                                 etp * 512:(etp + 1) * 512])
                    blks.append(wt)
                for sub in range(4):
                    et = etp * 4 + sub
                    ps = [pg2.tile([128, w], f32, tag="g2", name=f"pg2_{et}_{ci}")
                          for ci, (off, w) in enumerate(CT)]
                    for ft in range(FT):
                        wv = blks[ft // 2][:, (ft % 2) * 512 + sub * 128:
                                           (ft % 2) * 512 + (sub + 1) * 128]
                        for ci, (off, w) in enumerate(CT):
                            nc.tensor.matmul(ps[ci][:], wv, hbf[ft][:, off:off + w],
                                             start=(ft == 0), stop=(ft == FT - 1))
                    for ci, (off, w) in enumerate(CT):
                        ot = outs.tile([128, 512], bf16, tag="ot", name=f"ot_{et}_{ci}")
                        nc.vector.tensor_scalar(out=ot[:, 0:w], in0=ps[ci][:],
                                                scalar1=b2_sb[:, et:et + 1],
                                                scalar2=None, op0=ALU.add)
                        eng = nc.sync if (et + ci) % 2 == 0 else nc.gpsimd
                        eng.dma_start(
                            out=outT_d[et * 128:(et + 1) * 128, off:off + w],
                            in_=ot[:, 0:w])

    nc.compile()
    return nc


def _get_programs():
    if "l1" not in _programs:
        _programs["l1"] = _build_launch1()
    if "l2" not in _programs:
        _programs["l2"] = _build_launch2()
    return _programs["l1"], _programs["l2"]


def _expert_ffn_host(toks, w1e, b1e, w2e, b2e):
    """Exact host fallback for capacity overflow (rare)."""
    from scipy.special import erf
    h = toks @ w1e + b1e
    h = 0.5 * h * (1.0 + erf(h / np.float32(np.sqrt(2.0))))
    return h.astype(np.float32) @ w2e + b2e


def kernel(**inputs):
    import ml_dtypes

    l1, l2 = _get_programs()

    x = np.ascontiguousarray(np.asarray(inputs["x"], dtype=np.float32))        # (S,B,E)
    in_w = np.asarray(inputs["in_proj_w"], dtype=np.float32)                   # (3E,E)
    in_b = np.asarray(inputs["in_proj_b"], dtype=np.float32)
    out_w = np.asarray(inputs["out_proj_w"], dtype=np.float32)                 # (E,E)
    out_b = np.asarray(inputs["out_proj_b"], dtype=np.float32)
    gate_w = np.asarray(inputs["gate_w"], dtype=np.float32)                    # (NE,E)
    w1 = np.asarray(inputs["w1"], dtype=np.float32)                            # (NE,E,F)
    b1 = np.asarray(inputs["b1"], dtype=np.float32)
    w2 = np.asarray(inputs["w2"], dtype=np.float32)                            # (NE,F,E)
    b2 = np.asarray(inputs["b2"], dtype=np.float32)
    ln1_g = np.asarray(inputs["ln1_g"], dtype=np.float32)
    ln1_b = np.asarray(inputs["ln1_b"], dtype=np.float32)
    ln2_g = np.asarray(inputs["ln2_g"], dtype=np.float32)
    ln2_b = np.asarray(inputs["ln2_b"], dtype=np.float32)

    bf = ml_dtypes.bfloat16
    col = lambda v: np.ascontiguousarray(v.reshape(-1, 1))

    # fold LN1 gain into the QKV weights; LN1 bias into the QKV bias
    in_w_f = in_w * ln1_g[None, :]                 # (3E, E)
    in_b_f = in_b + in_w @ ln1_b                   # (3E,)

    # per-core head-group weight packs: pair-major [q 128 | k 128 | v 128]
    # head-group g covers heads 8g..8g+7 -> feature rows 512g..512(g+1)
    wqkv_g, bqkv_g, woT_g = [], [], []
    for g in range(2):
        fs = slice(512 * g, 512 * (g + 1))
        wq = in_w_f[0 * E:1 * E][fs]               # (512, E)
        wk = in_w_f[1 * E:2 * E][fs]
        wv = in_w_f[2 * E:3 * E][fs]
        pack = np.concatenate([wq.T, wk.T, wv.T], axis=1)   # (E, 1536)
        bias = np.zeros((8 * 128, 1), dtype=np.float32)
        bias[0:512, 0] = in_b_f[0 * E:1 * E][fs]
        bias[512:1024, 0] = in_b_f[1 * E:2 * E][fs]
        wqkv_g.append(np.ascontiguousarray(pack.astype(bf)))
        bqkv_g.append(bias)
        woT_g.append(np.ascontiguousarray(out_w[:, fs].T.astype(bf)))  # (512, E)

    sel2 = np.zeros((2, 128), dtype=np.float32)
    sel2[0, 0:64] = 1.0
    sel2[1, 64:128] = 1.0
    ident = np.eye(128, dtype=np.float32).astype(bf)

    # v-bias folds into a constant added on the host:
    # (o/den + bv_g) @ Wo_g.T summed over g  ->  + bv @ Wo.T (+ out_b)
    bv = in_b_f[2 * E:3 * E]
    const_out = out_b + bv @ out_w.T               # (E,)

    # ---- launch 1 ----
    xT_b = [np.ascontiguousarray(x[:, b, :].T.astype(bf)) for b in range(B)]
    in_maps1 = []
    for c in range(NCORES):
        b, g = divmod(c, 2)
        in_maps1.append({
            "xT": xT_b[b],
            "wqkv": wqkv_g[g],
            "bqkv": bqkv_g[g],
            "woT": woT_g[g],
            "sel2": sel2,
            "ident": ident,
        })
    res1 = run_bass_kernel_spmd(l1, in_maps1, list(range(NCORES)))

    # combine partials + residual -> x2 [E, S, B]; then LN2 + gating on host
    x2_all = np.empty((E, S, B), dtype=np.float32)
    for b in range(B):
        x2_all[:, :, b] = (x[:, b, :].T
                           + res1.results[2 * b]["outT"].astype(np.float32)
                           + res1.results[2 * b + 1]["outT"].astype(np.float32)
                           + const_out[:, None])
    x2_flat = x2_all.reshape(E, N)                 # token n = s*B + b

    mu = x2_flat.mean(axis=0)
    var = x2_flat.var(axis=0)
    h2_flat = ((x2_flat - mu) / np.sqrt(var + 1e-5)) * ln2_g[:, None] + ln2_b[:, None]
    h2_flat = h2_flat.astype(np.float32)

    # ---- host gating: softmax over NE logits, top-2 renormalized ----
    logits = gate_w @ h2_flat                      # (NE, N)
    logits -= logits.max(axis=0, keepdims=True)
    p = np.exp(logits)
    p /= p.sum(axis=0, keepdims=True)
    ar = np.arange(N)
    i1 = np.argmax(p, axis=0)
    v1 = p[i1, ar]
    pm = p.copy()
    pm[i1, ar] = -1.0
    i2 = np.argmax(pm, axis=0)
    v2 = p[i2, ar]
    gsum = v1 + v2
    gate1 = v1 / gsum
    gate2 = v2 / gsum

    h2_bf = h2_flat.astype(bf)
    idx_list, gates_list, ov_list = [], [], []
    in_maps2 = []
    for e in range(NE):
        sel = np.where((i1 == e) | (i2 == e))[0]
        ge = np.where(i1[sel] == e, gate1[sel], gate2[sel]).astype(np.float32)
        ov = None
        if len(sel) > C:
            ov = (sel[C:], ge[C:])
            sel, ge = sel[:C], ge[:C]
        idx_list.append(sel)
        gates_list.append(ge)
        ov_list.append(ov)
        toksT = np.zeros((E, C), dtype=bf)
        toksT[:, :len(sel)] = h2_bf[:, sel]
        in_maps2.append({
            "toksT": toksT,
            "w1": w1[e].astype(bf),
            "w2": w2[e].astype(bf),
            "b1": col(b1[e]),
            "b2": col(b2[e]),
        })
    res2 = run_bass_kernel_spmd(l2, in_maps2, list(range(NCORES)))

    # ---- combine ----
    out_flat = x2_flat
    for e in range(NE):
        sel, ge = idx_list[e], gates_list[e]
        out_flat[:, sel] += (res2.results[e]["outT"][:, :len(sel)].astype(np.float32)
                             * ge[None, :])
        if ov_list[e] is not None:
            osel, oge = ov_list[e]
            oo = _expert_ffn_host(h2_flat[:, osel].T, w1[e], b1[e], w2[e], b2[e])
            out_flat[:, osel] += oo.T * oge[None, :]

    return np.ascontiguousarray(
        out_flat.reshape(E, S, B).transpose(1, 2, 0)).astype(np.float32)
